# revision 1
# baseline (speedup 1.0000x reference)
"""GAT (2-layer graph attention + pair scoring) on 8 TRN2 NeuronCores.

Sharding: destination-node rows (4096/8=512 per core). Per layer, each core
computes Wh for its rows, scales by q=exp(0.2*e_dst), AllGathers the scaled
[N, nhid(+aux)] matrix, then computes masked attention for its 512 rows
against all 4096 sources. Pair scoring shards the 65536 pairs over cores and
gathers embeddings with indirect DMA.

Key algebra: exp(leaky_relu(s)) with s = e_src_i + e_dst_j factors as
  p_i * q_j * max(a_i*b_j, 1),  a=exp(.8 e_src), b=exp(.8 e_dst),
  p=exp(.2 e_src), q=exp(.2 e_dst)
and p_i cancels between softmax numerator and denominator. So the N^2 stage
needs NO transcendentals: one dual-op tensor_scalar (mult+max) and one
tensor_tensor (mask multiply) per tile. The softmax denominator rides along
as an extra q-column in the matmul's stationary operand.

Layer-1 AG payload: per-head blocks [Whq_h (64) | q_h] then 8 b columns.
Each head's attention matmul (lhsT = [Whq_h | q_h]) leaves numerators on PSUM
partitions 0..63 and the softmax denominator on partition 64. DVE copies with
shifted partition windows (verified on HW) assemble hcat^T; the denominators
hop partitions via one small SBUF->SBUF DMA.
"""

import sys

if "/opt/trn_rl_repo" not in sys.path:
    sys.path.insert(0, "/opt/trn_rl_repo")

import numpy as np
import ml_dtypes

import concourse.bass as bass
import concourse.bacc as bacc
import concourse.tile as tile
import concourse.mybir as mybir

BF16 = mybir.dt.bfloat16
F32 = mybir.dt.float32
I32 = mybir.dt.int32
I16 = mybir.dt.int16
AF = mybir.ActivationFunctionType
OP = mybir.AluOpType
AX = mybir.AxisListType

N, NFEAT, NHID, NHEADS = 4096, 512, 64, 8
P = 65536
NCORES = 8
R = N // NCORES          # rows (destination nodes) per core = 512
JT = N // 128            # source j-tiles = 32
PC = P // NCORES         # pairs per core = 8192
CH = PC // 128           # pair chunks = 64
HB = NHID + 1            # per-head AG1 block [Whq (64) | q] = 65
AG1C = NHEADS * HB + NHEADS   # 520 + 8 trailing b columns = 528
AG2C = NHID + 2          # [Whq2 (64) | q2 | b2] = 66

# heads whose mask-multiply runs on GPSIMD instead of DVE (load balance)
GP_HEADS = (3, 7)
GP_JT2 = 4               # in layer 2, every GP_JT2-th j-tile's mask-mul on gpsimd


def _build_nc(stage=99, iters=1):
    nc = bacc.Bacc("TRN2", target_bir_lowering=False, debug=False,
                   num_devices=NCORES)

    def inp(name, shape, dt):
        return nc.dram_tensor(name, shape, dt, kind="ExternalInput").ap()

    xT = inp("xT", [NFEAT, R], F32)            # x[rows].T  (feature-major)
    maskT = inp("maskT", [N, R], BF16)         # adj[rows].T (0/1)
    Wcat = inp("Wcat", [NFEAT, NHEADS * NHID], F32)
    Asrc = inp("Asrc", [NHEADS * NHID, NHEADS], F32)   # block-diag a_src
    Adst = inp("Adst", [NHEADS * NHID, NHEADS], F32)   # block-diag a_dst
    Wout = inp("Wout", [NHEADS * NHID, NHID], BF16)
    aout2 = inp("aout2", [NHID, 2], BF16)      # col0 = a_out[:64], col1 = a_out[64:]
    WsT = inp("WsT", [NHID, NHID], F32)        # W_score.T
    ident = inp("ident", [128, 128], F32)
    selbc = inp("selbc", [NHEADS, R], F32)     # selbc[h, m] = (m//64 == h)
    idx1 = inp("idx1", [128, CH], I32)   # idx[p, c] = pair index of slot p*CH+c
    idx2 = inp("idx2", [128, CH], I32)

    scores = nc.dram_tensor("scores", [128, CH], F32, kind="ExternalOutput").ap()

    rg = [list(range(NCORES))]

    with tile.TileContext(nc) as tc:
        with tc.tile_pool(name="sb", bufs=1) as sb, \
             tc.tile_pool(name="sbw", bufs=6) as sbw, \
             tc.tile_pool(name="ps", bufs=8, space="PSUM") as ps, \
             tc.tile_pool(name="dram", bufs=1, space="DRAM") as dram:

            for _it in range(iters):
                def pst(name):
                    return ps.tile([128, R], F32, tag="ps", name=name)

                # ---------- persistent loads ----------
                Wout_sb = sb.tile([128, 4, NHID], BF16, tag="Wout")
                nc.sync.dma_start(Wout_sb[:], Wout.rearrange("(k p) c -> p k c", p=128))
                aout2_sb = sb.tile([NHID, 2], BF16, tag="aout2")
                nc.sync.dma_start(aout2_sb[:], aout2[:])
                WsT_sb = sb.tile([NHID, NHID], F32, tag="WsT")
                nc.sync.dma_start(WsT_sb[:], WsT[:])
                ident_sb = sb.tile([128, 128], F32, tag="ident")
                nc.sync.dma_start(ident_sb[:], ident[:])
                selbc_sb = sb.tile([NHEADS, R], F32, tag="selbc")
                nc.sync.dma_start(selbc_sb[:], selbc[:])
                maskT_sb = sb.tile([128, JT, R], BF16, tag="maskT")
                nc.sync.dma_start(maskT_sb[:], maskT.rearrange("(j p) c -> p j c", p=128))
                idx1_sb = sb.tile([128, CH], I32, tag="idx1")
                nc.sync.dma_start(idx1_sb[:], idx1[:])
                idx2_sb = sb.tile([128, CH], I32, tag="idx2")
                nc.sync.dma_start(idx2_sb[:], idx2[:])
                ones_sb = sb.tile([65, 128], F32, tag="ones")
                nc.vector.memset(ones_sb[:], 1.0)

                abc_sb = sb.tile([128, NHEADS, R], BF16, tag="abc")
                ag1_sb = sb.tile([128, JT, AG1C], BF16, tag="ag1sb")
                bf_sb = sb.tile([128, JT, NHEADS], F32, tag="bf")
                hcatT_sb = sb.tile([128, 4, R], BF16, tag="hcatT")
                num_sb = sb.tile([128, 4, R], BF16, tag="num")
                xb_sb = sb.tile([128, 4, R], BF16, tag="xb")

                ag1_in = dram.tile([R, AG1C], BF16, tag="ag1in")
                ag1_out = dram.tile([N, AG1C], BF16, tag="ag1out", addr_space="Shared")

                # ---------- Phase A: local Wh / e / exps / AG1 payload ----------
                with tc.tile_pool(name="sbA", bufs=1) as sbA:
                    xT_sb = sbA.tile([128, 4, R], F32, tag="xT")
                    nc.sync.dma_start(xT_sb[:], xT.rearrange("(k p) c -> p k c", p=128))
                    Wcat_sb = sbA.tile([128, 4, NHEADS * NHID], F32, tag="Wcat")
                    nc.sync.dma_start(Wcat_sb[:],
                                      Wcat.rearrange("(k p) c -> p k c", p=128))
                    Asrc_sb = sbA.tile([128, 4, NHEADS], F32, tag="Asrc")
                    nc.sync.dma_start(Asrc_sb[:],
                                      Asrc.rearrange("(k p) c -> p k c", p=128))
                    Adst_sb = sbA.tile([128, 4, NHEADS], F32, tag="Adst")
                    nc.sync.dma_start(Adst_sb[:],
                                      Adst.rearrange("(k p) c -> p k c", p=128))

                    # Wh row-major [512_i, 512_hd] and WhT [512_hd, 512_i]
                    wh_sb = sbA.tile([128, 4, NHEADS * NHID], BF16, tag="wh")
                    whT_sb = sbA.tile([128, 4, R], F32, tag="whT")
                    for m in range(4):
                        wh_ps = pst(f"whps{m}")
                        for k in range(4):
                            nc.tensor.matmul(wh_ps[:],
                                             xT_sb[:, k, m * 128:(m + 1) * 128],
                                             Wcat_sb[:, k, :],
                                             start=(k == 0), stop=(k == 3))
                        nc.vector.tensor_copy(wh_sb[:, m, :], wh_ps[:])
                        whT_ps = pst(f"whTps{m}")
                        for k in range(4):
                            nc.tensor.matmul(whT_ps[:],
                                             Wcat_sb[:, k, m * 128:(m + 1) * 128],
                                             xT_sb[:, k, :],
                                             start=(k == 0), stop=(k == 3))
                        nc.vector.tensor_copy(whT_sb[:, m, :], whT_ps[:])

                    # abc[h] = exp(0.8*e_src_h) broadcast to all partitions:
                    # stride-0 lhsT makes every output partition identical
                    for h in range(NHEADS):
                        ebc_ps = pst(f"ebc{h}")
                        for k in range(4):
                            nc.tensor.matmul(
                                ebc_ps[:],
                                Asrc_sb[:, k, h:h + 1].to_broadcast([128, 128]),
                                whT_sb[:, k, :],
                                start=(k == 0), stop=(k == 3))
                        nc.scalar.activation(abc_sb[:, h, :], ebc_ps[:], AF.Exp,
                                             scale=0.8)

                    # e_dstT [512_i, 8] -> qT (f32, scalar use) and bT (bf16)
                    qT_sb = sbA.tile([128, 4, NHEADS], F32, tag="qT")
                    bT_sb = sbA.tile([128, 4, NHEADS], BF16, tag="bT")
                    for m in range(4):
                        ed_ps = pst(f"ed{m}")
                        for k in range(4):
                            nc.tensor.matmul(ed_ps[:, 0:NHEADS],
                                             whT_sb[:, k, m * 128:(m + 1) * 128],
                                             Adst_sb[:, k, :],
                                             start=(k == 0), stop=(k == 3))
                        nc.scalar.activation(qT_sb[:, m, :], ed_ps[:, 0:NHEADS],
                                             AF.Exp, scale=0.2)
                        nc.scalar.activation(bT_sb[:, m, :], ed_ps[:, 0:NHEADS],
                                             AF.Exp, scale=0.8)

                    # AG1 payload: per-head [Whq_h | q_h] blocks + 8 b columns
                    pay1_sb = sbA.tile([128, 4, AG1C], BF16, tag="pay1")
                    for m in range(4):
                        for h in range(NHEADS):
                            nc.vector.tensor_scalar(
                                pay1_sb[:, m, h * HB:h * HB + NHID],
                                wh_sb[:, m, h * NHID:(h + 1) * NHID],
                                qT_sb[:, m, h:h + 1], None, OP.mult)
                            nc.vector.tensor_copy(
                                pay1_sb[:, m, h * HB + NHID:h * HB + NHID + 1],
                                qT_sb[:, m, h:h + 1])
                        nc.vector.tensor_copy(pay1_sb[:, m, NHEADS * HB:AG1C],
                                              bT_sb[:, m, :])

                    nc.sync.dma_start(
                        ag1_in[:].rearrange("(m p) c -> p m c", p=128), pay1_sb[:])

                nc.gpsimd.collective_compute(
                    "AllGather", OP.bypass, replica_groups=rg,
                    ins=[ag1_in.opt()], outs=[ag1_out.opt()])
                nc.sync.dma_start(ag1_sb[:],
                                  ag1_out[:].rearrange("(j p) c -> p j c", p=128))
                nc.vector.tensor_copy(bf_sb[:], ag1_sb[:, :, NHEADS * HB:AG1C])

                if stage == 1:
                    dmy = sb.tile([128, CH], F32, tag="dmy", name="dmy1")
                    nc.vector.memset(dmy[:], 0.0)
                    nc.sync.dma_start(scores[:], dmy[:])
                    return nc

                # ---------- Phase B: layer-1 attention ----------
                # lhsT = [Whq_h | q_h]: numerators on psum p0..63, denom on p64
                hp_ps = [ps.tile([65, R], F32, tag="ps", name=f"hp{h}")
                         for h in range(NHEADS)]
                for j in range(JT):
                    for h in range(NHEADS):
                        t = sbw.tile([128, R], BF16, tag="t")
                        nc.vector.tensor_scalar(
                            t[:], abc_sb[:, h, :], bf_sb[:, j, h:h + 1],
                            1.0, OP.mult, OP.max)
                        t2 = sbw.tile([128, R], BF16, tag="t2")
                        eng = nc.gpsimd if h in GP_HEADS else nc.vector
                        eng.tensor_tensor(t2[:], t[:], maskT_sb[:, j, :], OP.mult)
                        nc.tensor.matmul(hp_ps[h][:],
                                         ag1_sb[:, j, h * HB:h * HB + NHID + 1],
                                         t2[:], start=(j == 0), stop=(j == JT - 1))


                if stage == 2:
                    dmy = sb.tile([128, CH], F32, tag="dmy", name="dmy2")
                    nc.vector.memset(dmy[:], 0.0)
                    nc.sync.dma_start(scores[:], dmy[:])
                    return nc
                # normalize + elu -> hcatT [512_hd, 512_i] bf16
                with tc.tile_pool(name="sbE", bufs=1) as sbE:
                    den_st = sbE.tile([65, NHEADS, R], F32, tag="den_st")
                    for h in range(NHEADS):
                        nc.vector.tensor_copy(den_st[64:65, h, :],
                                              hp_ps[h][64:65, :])
                        if h % 2 == 0:
                            nc.vector.tensor_copy(num_sb[0:64, h // 2, :],
                                                  hp_ps[h][0:64, :])
                        else:
                            nc.vector.tensor_copy(num_sb[64:128, h // 2, :],
                                                  hp_ps[h][0:64, :])
                    rin_sb = sbE.tile([NHEADS, R], F32, tag="rin")
                    nc.sync.dma_start(rin_sb[:], den_st[64:65, :, :])
                    rcp_sb = sbE.tile([NHEADS, R], F32, tag="rcp")
                    nc.vector.reciprocal(rcp_sb[:], rin_sb[:])

                    for g in range(4):
                        rbc_ps = pst(f"rbc{g}")
                        nc.tensor.matmul(rbc_ps[:],
                                         selbc_sb[:, g * 128:(g + 1) * 128],
                                         rcp_sb[:], start=True, stop=True)
                        nc.vector.tensor_tensor(xb_sb[:, g, :], num_sb[:, g, :],
                                                rbc_ps[:], OP.mult)
                    # elu(x) = relu(x) + exp(min(x,0)) - 1
                    tmin = sbE.tile([128, 4, R], BF16, tag="tmin")
                    nc.vector.tensor_scalar(tmin[:], xb_sb[:], 0.0, None, OP.min)
                    texp = sbE.tile([128, 4, R], BF16, tag="texp")
                    nc.scalar.activation(texp[:], tmin[:], AF.Exp)
                    trel = sbE.tile([128, 4, R], BF16, tag="trel")
                    nc.vector.tensor_scalar(trel[:], xb_sb[:], 0.0, 1.0, OP.max,
                                            OP.subtract)
                    nc.vector.tensor_tensor(hcatT_sb[:], texp[:], trel[:], OP.add)

                # ---------- Phase C: layer-2 ingredients + AG2 ----------
                ag2_in = dram.tile([R, AG2C], BF16, tag="ag2in")
                ag2_out = dram.tile([N, AG2C], BF16, tag="ag2out", addr_space="Shared")
                wh2T_sb = sb.tile([NHID, R], BF16, tag="wh2Tsb")
                a2bc_sb = sb.tile([128, R], BF16, tag="a2bcsb")
                with tc.tile_pool(name="sbC", bufs=1) as sbC:
                    wh2T_ps = ps.tile([NHID, R], F32, tag="ps", name="wh2T")
                    for k in range(4):
                        nc.tensor.matmul(wh2T_ps[:], Wout_sb[:, k, :],
                                         hcatT_sb[:, k, :],
                                         start=(k == 0), stop=(k == 3))
                    nc.vector.tensor_copy(wh2T_sb[:], wh2T_ps[:])

                    a2e_ps = pst("a2e")
                    nc.tensor.matmul(a2e_ps[:],
                                     aout2_sb[:, 0:1].to_broadcast([NHID, 128]),
                                     wh2T_sb[:], start=True, stop=True)
                    nc.scalar.activation(a2bc_sb[:], a2e_ps[:], AF.Exp, scale=0.8)

                    # AG2 payload block: [Whq2 (64) | q2 | b2]
                    pay2_sb = sbC.tile([128, 4, AG2C], BF16, tag="pay2")
                    for m in range(4):
                        wh2_ps = ps.tile([128, NHID], F32, tag="ps",
                                         name=f"wh2_{m}")
                        for k in range(4):
                            nc.tensor.matmul(wh2_ps[:],
                                             hcatT_sb[:, k, m * 128:(m + 1) * 128],
                                             Wout_sb[:, k, :],
                                             start=(k == 0), stop=(k == 3))
                        ed2_ps = ps.tile([128, 1], F32, tag="ps", name=f"ed2_{m}")
                        nc.tensor.matmul(ed2_ps[:],
                                         wh2T_sb[:, m * 128:(m + 1) * 128],
                                         aout2_sb[:, 1:2], start=True, stop=True)
                        q2f = sbC.tile([128, 4], F32, tag="q2f", bufs=4,
                                       name=f"q2f_{m}")
                        nc.scalar.activation(q2f[:, 0:1], ed2_ps[:], AF.Exp,
                                             scale=0.2)
                        nc.vector.tensor_copy(pay2_sb[:, m, NHID:NHID + 1],
                                              q2f[:, 0:1])
                        nc.scalar.activation(pay2_sb[:, m, NHID + 1:NHID + 2],
                                             ed2_ps[:], AF.Exp, scale=0.8)
                        nc.vector.tensor_scalar(pay2_sb[:, m, 0:NHID],
                                                wh2_ps[:], q2f[:, 0:1],
                                                None, OP.mult)
                    nc.sync.dma_start(
                        ag2_in[:].rearrange("(m p) c -> p m c", p=128), pay2_sb[:])

                nc.gpsimd.collective_compute(
                    "AllGather", OP.bypass, replica_groups=rg,
                    ins=[ag2_in.opt()], outs=[ag2_out.opt()])
                ag2_sb = sb.tile([128, JT, AG2C], BF16, tag="ag2sb")
                nc.sync.dma_start(ag2_sb[:],
                                  ag2_out[:].rearrange("(j p) c -> p j c", p=128))
                b2f_sb = sb.tile([128, JT, 1], F32, tag="b2f")
                nc.vector.tensor_copy(b2f_sb[:], ag2_sb[:, :, NHID + 1:NHID + 2])

                if stage == 3:
                    dmy = sb.tile([128, CH], F32, tag="dmy", name="dmy3")
                    nc.vector.memset(dmy[:], 0.0)
                    nc.sync.dma_start(scores[:], dmy[:])
                    return nc

                # ---------- Phase D: layer-2 attention ----------
                hp2_ps = ps.tile([65, R], F32, tag="ps", name="hp2")
                for j in range(JT):
                    t = sbw.tile([128, R], BF16, tag="t")
                    nc.vector.tensor_scalar(t[:], a2bc_sb[:], b2f_sb[:, j, 0:1],
                                            1.0, OP.mult, OP.max)
                    t2 = sbw.tile([128, R], BF16, tag="t2")
                    eng = nc.gpsimd if (j % GP_JT2 == GP_JT2 - 1) else nc.vector
                    eng.tensor_tensor(t2[:], t[:], maskT_sb[:, j, :], OP.mult)
                    nc.tensor.matmul(hp2_ps[:], ag2_sb[:, j, 0:NHID + 1], t2[:],
                                     start=(j == 0), stop=(j == JT - 1))


                if stage == 4:
                    dmy = sb.tile([128, CH], F32, tag="dmy", name="dmy4")
                    nc.vector.memset(dmy[:], 0.0)
                    nc.sync.dma_start(scores[:], dmy[:])
                    return nc
                hfT_sb = sb.tile([NHID, R], F32, tag="hfT")
                with tc.tile_pool(name="sbD", bufs=1) as sbD:
                    rcp2_sb = sbD.tile([65, R], F32, tag="rcp2")
                    nc.vector.reciprocal(rcp2_sb[64:65, :], hp2_ps[64:65, :])
                    rbc2_ps = ps.tile([NHID, R], F32, tag="ps", name="rbc2")
                    nc.tensor.matmul(rbc2_ps[:], ones_sb[64:65, 0:NHID],
                                     rcp2_sb[64:65, :], start=True, stop=True)
                    num2_sb = sbD.tile([NHID, R], F32, tag="num2")
                    nc.vector.tensor_copy(num2_sb[:], hp2_ps[0:NHID, :])
                    xn2_sb = sbD.tile([NHID, R], F32, tag="xn2")
                    nc.vector.tensor_tensor(xn2_sb[:], num2_sb[:],
                                            rbc2_ps[:], OP.mult)
                    # elu in f32
                    tmin2 = sbD.tile([NHID, R], F32, tag="tmin2")
                    nc.vector.tensor_scalar(tmin2[:], xn2_sb[:], 0.0, None, OP.min)
                    texp2 = sbD.tile([NHID, R], F32, tag="texp2")
                    nc.scalar.activation(texp2[:], tmin2[:], AF.Exp)
                    trel2 = sbD.tile([NHID, R], F32, tag="trel2")
                    nc.vector.tensor_scalar(trel2[:], xn2_sb[:], 0.0, 1.0, OP.max,
                                            OP.subtract)
                    nc.vector.tensor_tensor(hfT_sb[:], texp2[:], trel2[:], OP.add)

                # ---------- Phase E: H2 = h @ Ws^T, transpose h, AG3 ----------
                ag3a_in = dram.tile([R, NHID], F32, tag="ag3ain")
                ag3a_out = dram.tile([N, NHID], F32, tag="ag3aout",
                                     addr_space="Shared")
                ag3b_in = dram.tile([R, NHID], F32, tag="ag3bin")
                ag3b_out = dram.tile([N, NHID], F32, tag="ag3bout",
                                     addr_space="Shared")
                with tc.tile_pool(name="sbF", bufs=1) as sbF:
                    ag3a_sb = sbF.tile([128, 4, NHID], F32, tag="ag3a")
                    ag3b_sb = sbF.tile([128, 4, NHID], F32, tag="ag3b")
                    for m in range(4):
                        h2_ps = ps.tile([128, NHID], F32, tag="ps", name=f"h2_{m}")
                        nc.tensor.matmul(h2_ps[:], hfT_sb[:, m * 128:(m + 1) * 128],
                                         WsT_sb[:], start=True, stop=True)
                        nc.vector.tensor_copy(ag3b_sb[:, m, :], h2_ps[:])
                        hf_ps = ps.tile([128, NHID], F32, tag="ps", name=f"hf_{m}")
                        nc.tensor.transpose(hf_ps[:],
                                            hfT_sb[:, m * 128:(m + 1) * 128],
                                            ident_sb[0:NHID, 0:NHID])
                        nc.vector.tensor_copy(ag3a_sb[:, m, :], hf_ps[:])

                    nc.sync.dma_start(
                        ag3a_in[:].rearrange("(m p) c -> p m c", p=128), ag3a_sb[:])
                    nc.sync.dma_start(
                        ag3b_in[:].rearrange("(m p) c -> p m c", p=128), ag3b_sb[:])
                    nc.gpsimd.collective_compute(
                        "AllGather", OP.bypass, replica_groups=rg,
                        ins=[ag3a_in.opt()], outs=[ag3a_out.opt()])
                    nc.gpsimd.collective_compute(
                        "AllGather", OP.bypass, replica_groups=rg,
                        ins=[ag3b_in.opt()], outs=[ag3b_out.opt()])


                    if stage == 5:
                        dmy = sbF.tile([128, CH], F32, tag="dmy", name="dmy5")
                        nc.vector.memset(dmy[:], 0.0)
                        nc.sync.dma_start(scores[:], dmy[:])
                        return nc
                    # ---------- Phase F: pair gather + bilinear score ----------
                    # HW indirect DMA honors one index per partition: 64 calls
                    # of 128-row gathers per table, pipelined on the Q7/SDMA path
                    g1_sb = sbF.tile([128, CH, NHID], F32, tag="g1")
                    g2_sb = sbF.tile([128, CH, NHID], F32, tag="g2")
                    for c in range(CH):
                        nc.gpsimd.indirect_dma_start(
                            out=g1_sb[:, c, :], out_offset=None, in_=ag3a_out[:],
                            in_offset=bass.IndirectOffsetOnAxis(
                                ap=idx1_sb[:, c:c + 1], axis=0))
                        nc.gpsimd.indirect_dma_start(
                            out=g2_sb[:, c, :], out_offset=None, in_=ag3b_out[:],
                            in_offset=bass.IndirectOffsetOnAxis(
                                ap=idx2_sb[:, c:c + 1], axis=0))

                    if stage == 6:
                        dmy = sbF.tile([128, CH], F32, tag="dmy", name="dmy6")
                        nc.vector.memset(dmy[:], 0.0)
                        nc.sync.dma_start(scores[:], dmy[:])
                        return nc
                    prod_sb = sbF.tile([128, CH, NHID], F32, tag="prod")
                    nc.vector.tensor_tensor(prod_sb[:], g1_sb[:], g2_sb[:], OP.mult)
                    sc_sb = sbF.tile([128, CH], F32, tag="sc")
                    nc.vector.tensor_reduce(sc_sb[:], prod_sb[:], AX.X, OP.add)
                    nc.sync.dma_start(scores[:], sc_sb[:])

    return nc


def _make_in_maps(x, adj, W_heads, a_heads, W_out, a_out, W_score,
                  pair1_idx, pair2_idx):
    bf = ml_dtypes.bfloat16
    x = np.asarray(x, dtype=np.float32)
    adj = np.asarray(adj, dtype=np.float32)
    W_heads = np.asarray(W_heads, dtype=np.float32)
    a_heads = np.asarray(a_heads, dtype=np.float32)
    W_out = np.asarray(W_out, dtype=np.float32)
    a_out = np.asarray(a_out, dtype=np.float32)
    W_score = np.asarray(W_score, dtype=np.float32)
    pair1_idx = np.asarray(pair1_idx, dtype=np.int32)
    pair2_idx = np.asarray(pair2_idx, dtype=np.int32)

    Wcat = np.concatenate([W_heads[h] for h in range(NHEADS)], axis=1)
    Wcat = np.ascontiguousarray(Wcat, dtype=np.float32)
    Asrc = np.zeros((NHEADS * NHID, NHEADS), dtype=np.float32)
    Adst = np.zeros((NHEADS * NHID, NHEADS), dtype=np.float32)
    for h in range(NHEADS):
        Asrc[h * NHID:(h + 1) * NHID, h] = a_heads[h, :NHID]
        Adst[h * NHID:(h + 1) * NHID, h] = a_heads[h, NHID:]
    Wout_bf = W_out.astype(bf)
    aout2 = np.stack([a_out[:NHID], a_out[NHID:]], axis=1).astype(bf)
    WsT = np.ascontiguousarray(W_score.T, dtype=np.float32)
    ident = np.eye(128, dtype=np.float32)
    selbc = np.zeros((NHEADS, R), dtype=np.float32)
    for h in range(NHEADS):
        selbc[h, h * NHID:(h + 1) * NHID] = 1.0

    in_maps = []
    for c in range(NCORES):
        rows = slice(c * R, (c + 1) * R)
        in_maps.append(dict(
            xT=np.ascontiguousarray(x[rows].T),
            maskT=np.ascontiguousarray(adj[rows].T).astype(bf),
            Wcat=Wcat, Asrc=Asrc, Adst=Adst, Wout=Wout_bf, aout2=aout2,
            WsT=WsT, ident=ident, selbc=selbc,
            idx1=np.ascontiguousarray(
                pair1_idx[c * PC:(c + 1) * PC].reshape(128, CH)),
            idx2=np.ascontiguousarray(
                pair2_idx[c * PC:(c + 1) * PC].reshape(128, CH)),
        ))
    return in_maps


_CACHE = {}


def _get_compiled(stage=99, iters=1):
    key = f"nc{stage}_{iters}"
    if key not in _CACHE:
        nc = _build_nc(stage, iters)
        nc.compile()
        _CACHE[key] = nc
    return _CACHE[key]


def kernel(**inputs):
    from concourse.bass_utils import run_bass_kernel_spmd

    nc = _get_compiled()
    in_maps = _make_in_maps(**inputs)
    res = run_bass_kernel_spmd(nc, in_maps, core_ids=list(range(NCORES)))
    out = np.concatenate(
        [np.asarray(res.results[c]["scores"], dtype=np.float32).reshape(PC)
         for c in range(NCORES)])
    return out



# revision 14
# speedup vs baseline: 7.8533x; 7.8533x over previous
"""GAT (2-layer graph attention + pair scoring) on 8 TRN2 NeuronCores.

Sharding: destination-node rows (4096/8=512 per core). Per layer, each core
computes Wh for its rows, scales by q=exp(0.2*e_dst), AllGathers the scaled
[N, nhid(+aux)] matrix, then computes masked attention for its 512 rows
against all 4096 sources. Pair scoring shards the 65536 pairs over cores and
gathers embeddings with indirect DMA.

Key algebra: exp(leaky_relu(s)) with s = e_src_i + e_dst_j factors as
  p_i * q_j * max(a_i*b_j, 1),  a=exp(.8 e_src), b=exp(.8 e_dst),
  p=exp(.2 e_src), q=exp(.2 e_dst)
and p_i cancels between softmax numerator and denominator. So the N^2 stage
needs NO transcendentals: one dual-op tensor_scalar (mult+max) and one
tensor_tensor (mask multiply) per tile. The softmax denominator rides along
as an extra q-column in the matmul's stationary operand.

Layer-1 AG payload: per-head blocks [Whq_h (64) | q_h] then 8 b columns.
Each head's attention matmul (lhsT = [Whq_h | q_h]) leaves numerators on PSUM
partitions 0..63 and the softmax denominator on partition 64. DVE copies with
shifted partition windows (verified on HW) assemble hcat^T; the denominators
hop partitions via one small SBUF->SBUF DMA.
"""

import sys

if "/opt/trn_rl_repo" not in sys.path:
    sys.path.insert(0, "/opt/trn_rl_repo")

import numpy as np
import ml_dtypes

import concourse.bass as bass
import concourse.bacc as bacc
import concourse.tile as tile
import concourse.mybir as mybir

BF16 = mybir.dt.bfloat16
F32 = mybir.dt.float32
I32 = mybir.dt.int32
I16 = mybir.dt.int16
AF = mybir.ActivationFunctionType
OP = mybir.AluOpType
AX = mybir.AxisListType

N, NFEAT, NHID, NHEADS = 4096, 512, 64, 8
P = 65536
NCORES = 8
R = N // NCORES          # rows (destination nodes) per core = 512
JT = N // 128            # source j-tiles = 32
PC = P // NCORES         # pairs per core = 8192
CH = PC // 128           # pair chunks = 64
HB = NHID + 1            # per-head AG1 block [Whq (64) | q] = 65
AG1C = NHEADS * HB + NHEADS   # 520 + 8 trailing b columns = 528
AG2C = NHID + 2          # [Whq2 (64) | q2 | b2] = 66
FW = 4864                # f32 blob columns (xT|Wcat|Asrc|Adst|WsT|ident|selbc)
BW = 16642               # bf16 blob columns (maskT|Wout|aout2)

# heads whose mask-multiply runs on GPSIMD instead of DVE (load balance)
GP_HEADS = (3, 7)
GP_JT2 = 4               # in layer 2, every GP_JT2-th j-tile's mask-mul on gpsimd


def _build_nc(stage=99, iters=1):
    nc = bacc.Bacc("TRN2", target_bir_lowering=False, debug=False,
                   num_devices=NCORES)

    def inp(name, shape, dt):
        return nc.dram_tensor(name, shape, dt, kind="ExternalInput").ap()

    # All inputs packed into 3 blobs (per-call dispatch cost scales with the
    # number of kernel I/O tensors). Column offsets must match _make_in_maps.
    blobF = inp("blobF", [128, FW], F32)
    blobB = inp("blobB", [128, BW], BF16)
    blobI = inp("blobI", [128, 2 * CH], I32)

    xT = blobF[:, 0:2048].rearrange("p (k c) -> p k c", k=4)
    Wcat = blobF[:, 2048:4096].rearrange("p (k c) -> p k c", k=4)
    Asrc = blobF[:, 4096:4128].rearrange("p (k c) -> p k c", k=4)
    Adst = blobF[:, 4128:4160].rearrange("p (k c) -> p k c", k=4)
    WsT = blobF[0:NHID, 4160:4224]
    ident = blobF[:, 4224:4352]
    selbc = blobF[0:NHEADS, 4352:4864]
    maskT = blobB[:, 0:16384].rearrange("p (j c) -> p j c", j=JT)
    Wout = blobB[:, 16384:16640].rearrange("p (k c) -> p k c", k=4)
    aout2 = blobB[0:NHID, 16640:16642]
    idx1 = blobI[:, 0:CH]
    idx2 = blobI[:, CH:2 * CH]

    scores = nc.dram_tensor("scores", [128, CH], F32, kind="ExternalOutput").ap()

    rg = [list(range(NCORES))]

    with tile.TileContext(nc) as tc:
        with tc.tile_pool(name="sb", bufs=1) as sb, \
             tc.tile_pool(name="sbw", bufs=6) as sbw, \
             tc.tile_pool(name="ps", bufs=8, space="PSUM") as ps, \
             tc.tile_pool(name="dram", bufs=1, space="DRAM") as dram:

            for _it in range(iters):
                def pst(name):
                    return ps.tile([128, R], F32, tag="ps", name=name)

                # ---------- persistent loads ----------
                Wout_sb = sb.tile([128, 4, NHID], BF16, tag="Wout")
                nc.sync.dma_start(Wout_sb[:], Wout)
                aout2_sb = sb.tile([NHID, 2], BF16, tag="aout2")
                nc.sync.dma_start(aout2_sb[:], aout2)
                WsT_sb = sb.tile([NHID, NHID], F32, tag="WsT")
                nc.sync.dma_start(WsT_sb[:], WsT)
                ident_sb = sb.tile([128, 128], F32, tag="ident")
                nc.sync.dma_start(ident_sb[:], ident)
                selbc_sb = sb.tile([NHEADS, R], F32, tag="selbc")
                nc.sync.dma_start(selbc_sb[:], selbc)
                maskT_sb = sb.tile([128, JT, R], BF16, tag="maskT")
                nc.sync.dma_start(maskT_sb[:], maskT)
                idx1_sb = sb.tile([128, CH], I32, tag="idx1")
                nc.sync.dma_start(idx1_sb[:], idx1)
                idx2_sb = sb.tile([128, CH], I32, tag="idx2")
                nc.sync.dma_start(idx2_sb[:], idx2)
                ones_sb = sb.tile([65, 128], F32, tag="ones")
                nc.vector.memset(ones_sb[:], 1.0)

                abc_sb = sb.tile([128, NHEADS, R], BF16, tag="abc")
                ag1_sb = sb.tile([128, JT, AG1C], BF16, tag="ag1sb")
                bf_sb = sb.tile([128, JT, NHEADS], F32, tag="bf")
                hcatT_sb = sb.tile([128, 4, R], BF16, tag="hcatT")
                num_sb = sb.tile([128, 4, R], BF16, tag="num")
                xb_sb = sb.tile([128, 4, R], BF16, tag="xb")

                ag1_in = dram.tile([R, AG1C], BF16, tag="ag1in")
                ag1_out = dram.tile([N, AG1C], BF16, tag="ag1out", addr_space="Shared")

                # ---------- Phase A: local Wh / e / exps / AG1 payload ----------
                with tc.tile_pool(name="sbA", bufs=1) as sbA:
                    xT_sb = sbA.tile([128, 4, R], F32, tag="xT")
                    nc.sync.dma_start(xT_sb[:], xT)
                    Wcat_sb = sbA.tile([128, 4, NHEADS * NHID], F32, tag="Wcat")
                    nc.sync.dma_start(Wcat_sb[:], Wcat)
                    Asrc_sb = sbA.tile([128, 4, NHEADS], F32, tag="Asrc")
                    nc.sync.dma_start(Asrc_sb[:], Asrc)
                    Adst_sb = sbA.tile([128, 4, NHEADS], F32, tag="Adst")
                    nc.sync.dma_start(Adst_sb[:], Adst)

                    # Wh row-major [512_i, 512_hd] and WhT [512_hd, 512_i]
                    wh_sb = sbA.tile([128, 4, NHEADS * NHID], BF16, tag="wh")
                    whT_sb = sbA.tile([128, 4, R], F32, tag="whT")
                    for m in range(4):
                        wh_ps = pst(f"whps{m}")
                        for k in range(4):
                            nc.tensor.matmul(wh_ps[:],
                                             xT_sb[:, k, m * 128:(m + 1) * 128],
                                             Wcat_sb[:, k, :],
                                             start=(k == 0), stop=(k == 3))
                        nc.vector.tensor_copy(wh_sb[:, m, :], wh_ps[:])
                        whT_ps = pst(f"whTps{m}")
                        for k in range(4):
                            nc.tensor.matmul(whT_ps[:],
                                             Wcat_sb[:, k, m * 128:(m + 1) * 128],
                                             xT_sb[:, k, :],
                                             start=(k == 0), stop=(k == 3))
                        nc.vector.tensor_copy(whT_sb[:, m, :], whT_ps[:])

                    # abc[h] = exp(0.8*e_src_h) broadcast to all partitions:
                    # stride-0 lhsT makes every output partition identical
                    for h in range(NHEADS):
                        ebc_ps = pst(f"ebc{h}")
                        for k in range(4):
                            nc.tensor.matmul(
                                ebc_ps[:],
                                Asrc_sb[:, k, h:h + 1].to_broadcast([128, 128]),
                                whT_sb[:, k, :],
                                start=(k == 0), stop=(k == 3))
                        nc.scalar.activation(abc_sb[:, h, :], ebc_ps[:], AF.Exp,
                                             scale=0.8)

                    # e_dstT [512_i, 8] -> qT (f32, scalar use) and bT (bf16)
                    qT_sb = sbA.tile([128, 4, NHEADS], F32, tag="qT")
                    bT_sb = sbA.tile([128, 4, NHEADS], BF16, tag="bT")
                    for m in range(4):
                        ed_ps = pst(f"ed{m}")
                        for k in range(4):
                            nc.tensor.matmul(ed_ps[:, 0:NHEADS],
                                             whT_sb[:, k, m * 128:(m + 1) * 128],
                                             Adst_sb[:, k, :],
                                             start=(k == 0), stop=(k == 3))
                        nc.scalar.activation(qT_sb[:, m, :], ed_ps[:, 0:NHEADS],
                                             AF.Exp, scale=0.2)
                        nc.scalar.activation(bT_sb[:, m, :], ed_ps[:, 0:NHEADS],
                                             AF.Exp, scale=0.8)

                    # AG1 payload: per-head [Whq_h | q_h] blocks + 8 b columns
                    pay1_sb = sbA.tile([128, 4, AG1C], BF16, tag="pay1")
                    for m in range(4):
                        for h in range(NHEADS):
                            nc.vector.tensor_scalar(
                                pay1_sb[:, m, h * HB:h * HB + NHID],
                                wh_sb[:, m, h * NHID:(h + 1) * NHID],
                                qT_sb[:, m, h:h + 1], None, OP.mult)
                            nc.vector.tensor_copy(
                                pay1_sb[:, m, h * HB + NHID:h * HB + NHID + 1],
                                qT_sb[:, m, h:h + 1])
                        nc.vector.tensor_copy(pay1_sb[:, m, NHEADS * HB:AG1C],
                                              bT_sb[:, m, :])

                    nc.sync.dma_start(
                        ag1_in[:].rearrange("(m p) c -> p m c", p=128), pay1_sb[:])

                nc.gpsimd.collective_compute(
                    "AllGather", OP.bypass, replica_groups=rg,
                    ins=[ag1_in.opt()], outs=[ag1_out.opt()])
                nc.sync.dma_start(ag1_sb[:],
                                  ag1_out[:].rearrange("(j p) c -> p j c", p=128))
                nc.vector.tensor_copy(bf_sb[:], ag1_sb[:, :, NHEADS * HB:AG1C])

                if stage == 1:
                    dmy = sb.tile([128, CH], F32, tag="dmy", name="dmy1")
                    nc.vector.memset(dmy[:], 0.0)
                    nc.sync.dma_start(scores[:], dmy[:])
                    return nc

                # ---------- Phase B: layer-1 attention ----------
                # lhsT = [Whq_h | q_h]: numerators on psum p0..63, denom on p64
                hp_ps = [ps.tile([65, R], F32, tag="ps", name=f"hp{h}")
                         for h in range(NHEADS)]
                for j in range(JT):
                    for h in range(NHEADS):
                        t = sbw.tile([128, R], BF16, tag="t")
                        nc.vector.tensor_scalar(
                            t[:], abc_sb[:, h, :], bf_sb[:, j, h:h + 1],
                            1.0, OP.mult, OP.max)
                        t2 = sbw.tile([128, R], BF16, tag="t2")
                        eng = nc.gpsimd if h in GP_HEADS else nc.vector
                        eng.tensor_tensor(t2[:], t[:], maskT_sb[:, j, :], OP.mult)
                        nc.tensor.matmul(hp_ps[h][:],
                                         ag1_sb[:, j, h * HB:h * HB + NHID + 1],
                                         t2[:], start=(j == 0), stop=(j == JT - 1))


                if stage == 2:
                    dmy = sb.tile([128, CH], F32, tag="dmy", name="dmy2")
                    nc.vector.memset(dmy[:], 0.0)
                    nc.sync.dma_start(scores[:], dmy[:])
                    return nc
                # normalize + elu -> hcatT [512_hd, 512_i] bf16
                with tc.tile_pool(name="sbE", bufs=1) as sbE:
                    den_st = sbE.tile([65, NHEADS, R], F32, tag="den_st")
                    for h in range(NHEADS):
                        nc.vector.tensor_copy(den_st[64:65, h, :],
                                              hp_ps[h][64:65, :])
                        if h % 2 == 0:
                            nc.vector.tensor_copy(num_sb[0:64, h // 2, :],
                                                  hp_ps[h][0:64, :])
                        else:
                            nc.vector.tensor_copy(num_sb[64:128, h // 2, :],
                                                  hp_ps[h][0:64, :])
                    rin_sb = sbE.tile([NHEADS, R], F32, tag="rin")
                    nc.sync.dma_start(rin_sb[:], den_st[64:65, :, :])
                    rcp_sb = sbE.tile([NHEADS, R], F32, tag="rcp")
                    nc.vector.reciprocal(rcp_sb[:], rin_sb[:])

                    for g in range(4):
                        rbc_ps = pst(f"rbc{g}")
                        nc.tensor.matmul(rbc_ps[:],
                                         selbc_sb[:, g * 128:(g + 1) * 128],
                                         rcp_sb[:], start=True, stop=True)
                        nc.vector.tensor_tensor(xb_sb[:, g, :], num_sb[:, g, :],
                                                rbc_ps[:], OP.mult)
                    # elu(x) = relu(x) + exp(min(x,0)) - 1
                    tmin = sbE.tile([128, 4, R], BF16, tag="tmin")
                    nc.vector.tensor_scalar(tmin[:], xb_sb[:], 0.0, None, OP.min)
                    texp = sbE.tile([128, 4, R], BF16, tag="texp")
                    nc.scalar.activation(texp[:], tmin[:], AF.Exp)
                    trel = sbE.tile([128, 4, R], BF16, tag="trel")
                    nc.vector.tensor_scalar(trel[:], xb_sb[:], 0.0, 1.0, OP.max,
                                            OP.subtract)
                    nc.vector.tensor_tensor(hcatT_sb[:], texp[:], trel[:], OP.add)

                # ---------- Phase C: layer-2 ingredients + AG2 ----------
                ag2_in = dram.tile([R, AG2C], BF16, tag="ag2in")
                ag2_out = dram.tile([N, AG2C], BF16, tag="ag2out", addr_space="Shared")
                wh2T_sb = sb.tile([NHID, R], BF16, tag="wh2Tsb")
                a2bc_sb = sb.tile([128, R], BF16, tag="a2bcsb")
                with tc.tile_pool(name="sbC", bufs=1) as sbC:
                    wh2T_ps = ps.tile([NHID, R], F32, tag="ps", name="wh2T")
                    for k in range(4):
                        nc.tensor.matmul(wh2T_ps[:], Wout_sb[:, k, :],
                                         hcatT_sb[:, k, :],
                                         start=(k == 0), stop=(k == 3))
                    nc.vector.tensor_copy(wh2T_sb[:], wh2T_ps[:])

                    a2e_ps = pst("a2e")
                    nc.tensor.matmul(a2e_ps[:],
                                     aout2_sb[:, 0:1].to_broadcast([NHID, 128]),
                                     wh2T_sb[:], start=True, stop=True)
                    nc.scalar.activation(a2bc_sb[:], a2e_ps[:], AF.Exp, scale=0.8)

                    # AG2 payload block: [Whq2 (64) | q2 | b2]
                    pay2_sb = sbC.tile([128, 4, AG2C], BF16, tag="pay2")
                    for m in range(4):
                        wh2_ps = ps.tile([128, NHID], F32, tag="ps",
                                         name=f"wh2_{m}")
                        for k in range(4):
                            nc.tensor.matmul(wh2_ps[:],
                                             hcatT_sb[:, k, m * 128:(m + 1) * 128],
                                             Wout_sb[:, k, :],
                                             start=(k == 0), stop=(k == 3))
                        ed2_ps = ps.tile([128, 1], F32, tag="ps", name=f"ed2_{m}")
                        nc.tensor.matmul(ed2_ps[:],
                                         wh2T_sb[:, m * 128:(m + 1) * 128],
                                         aout2_sb[:, 1:2], start=True, stop=True)
                        q2f = sbC.tile([128, 4], F32, tag="q2f", bufs=4,
                                       name=f"q2f_{m}")
                        nc.scalar.activation(q2f[:, 0:1], ed2_ps[:], AF.Exp,
                                             scale=0.2)
                        nc.vector.tensor_copy(pay2_sb[:, m, NHID:NHID + 1],
                                              q2f[:, 0:1])
                        nc.scalar.activation(pay2_sb[:, m, NHID + 1:NHID + 2],
                                             ed2_ps[:], AF.Exp, scale=0.8)
                        nc.vector.tensor_scalar(pay2_sb[:, m, 0:NHID],
                                                wh2_ps[:], q2f[:, 0:1],
                                                None, OP.mult)
                    nc.sync.dma_start(
                        ag2_in[:].rearrange("(m p) c -> p m c", p=128), pay2_sb[:])

                nc.gpsimd.collective_compute(
                    "AllGather", OP.bypass, replica_groups=rg,
                    ins=[ag2_in.opt()], outs=[ag2_out.opt()])
                ag2_sb = sb.tile([128, JT, AG2C], BF16, tag="ag2sb")
                nc.sync.dma_start(ag2_sb[:],
                                  ag2_out[:].rearrange("(j p) c -> p j c", p=128))
                b2f_sb = sb.tile([128, JT, 1], F32, tag="b2f")
                nc.vector.tensor_copy(b2f_sb[:], ag2_sb[:, :, NHID + 1:NHID + 2])

                if stage == 3:
                    dmy = sb.tile([128, CH], F32, tag="dmy", name="dmy3")
                    nc.vector.memset(dmy[:], 0.0)
                    nc.sync.dma_start(scores[:], dmy[:])
                    return nc

                # ---------- Phase D: layer-2 attention ----------
                hp2_ps = ps.tile([65, R], F32, tag="ps", name="hp2")
                for j in range(JT):
                    t = sbw.tile([128, R], BF16, tag="t")
                    nc.vector.tensor_scalar(t[:], a2bc_sb[:], b2f_sb[:, j, 0:1],
                                            1.0, OP.mult, OP.max)
                    t2 = sbw.tile([128, R], BF16, tag="t2")
                    eng = nc.gpsimd if (j % GP_JT2 == GP_JT2 - 1) else nc.vector
                    eng.tensor_tensor(t2[:], t[:], maskT_sb[:, j, :], OP.mult)
                    nc.tensor.matmul(hp2_ps[:], ag2_sb[:, j, 0:NHID + 1], t2[:],
                                     start=(j == 0), stop=(j == JT - 1))


                if stage == 4:
                    dmy = sb.tile([128, CH], F32, tag="dmy", name="dmy4")
                    nc.vector.memset(dmy[:], 0.0)
                    nc.sync.dma_start(scores[:], dmy[:])
                    return nc
                hfT_sb = sb.tile([NHID, R], F32, tag="hfT")
                with tc.tile_pool(name="sbD", bufs=1) as sbD:
                    rcp2_sb = sbD.tile([65, R], F32, tag="rcp2")
                    nc.vector.reciprocal(rcp2_sb[64:65, :], hp2_ps[64:65, :])
                    rbc2_ps = ps.tile([NHID, R], F32, tag="ps", name="rbc2")
                    nc.tensor.matmul(rbc2_ps[:], ones_sb[64:65, 0:NHID],
                                     rcp2_sb[64:65, :], start=True, stop=True)
                    num2_sb = sbD.tile([NHID, R], F32, tag="num2")
                    nc.vector.tensor_copy(num2_sb[:], hp2_ps[0:NHID, :])
                    xn2_sb = sbD.tile([NHID, R], F32, tag="xn2")
                    nc.vector.tensor_tensor(xn2_sb[:], num2_sb[:],
                                            rbc2_ps[:], OP.mult)
                    # elu in f32
                    tmin2 = sbD.tile([NHID, R], F32, tag="tmin2")
                    nc.vector.tensor_scalar(tmin2[:], xn2_sb[:], 0.0, None, OP.min)
                    texp2 = sbD.tile([NHID, R], F32, tag="texp2")
                    nc.scalar.activation(texp2[:], tmin2[:], AF.Exp)
                    trel2 = sbD.tile([NHID, R], F32, tag="trel2")
                    nc.vector.tensor_scalar(trel2[:], xn2_sb[:], 0.0, 1.0, OP.max,
                                            OP.subtract)
                    nc.vector.tensor_tensor(hfT_sb[:], texp2[:], trel2[:], OP.add)

                # ---------- Phase E: H2 = h @ Ws^T, transpose h, AG3 ----------
                ag3a_in = dram.tile([R, NHID], F32, tag="ag3ain")
                ag3a_out = dram.tile([N, NHID], F32, tag="ag3aout",
                                     addr_space="Shared")
                ag3b_in = dram.tile([R, NHID], F32, tag="ag3bin")
                ag3b_out = dram.tile([N, NHID], F32, tag="ag3bout",
                                     addr_space="Shared")
                with tc.tile_pool(name="sbF", bufs=1) as sbF:
                    ag3a_sb = sbF.tile([128, 4, NHID], F32, tag="ag3a")
                    ag3b_sb = sbF.tile([128, 4, NHID], F32, tag="ag3b")
                    for m in range(4):
                        h2_ps = ps.tile([128, NHID], F32, tag="ps", name=f"h2_{m}")
                        nc.tensor.matmul(h2_ps[:], hfT_sb[:, m * 128:(m + 1) * 128],
                                         WsT_sb[:], start=True, stop=True)
                        nc.vector.tensor_copy(ag3b_sb[:, m, :], h2_ps[:])
                        hf_ps = ps.tile([128, NHID], F32, tag="ps", name=f"hf_{m}")
                        nc.tensor.transpose(hf_ps[:],
                                            hfT_sb[:, m * 128:(m + 1) * 128],
                                            ident_sb[0:NHID, 0:NHID])
                        nc.vector.tensor_copy(ag3a_sb[:, m, :], hf_ps[:])

                    nc.sync.dma_start(
                        ag3a_in[:].rearrange("(m p) c -> p m c", p=128), ag3a_sb[:])
                    nc.sync.dma_start(
                        ag3b_in[:].rearrange("(m p) c -> p m c", p=128), ag3b_sb[:])
                    nc.gpsimd.collective_compute(
                        "AllGather", OP.bypass, replica_groups=rg,
                        ins=[ag3a_in.opt()], outs=[ag3a_out.opt()])
                    nc.gpsimd.collective_compute(
                        "AllGather", OP.bypass, replica_groups=rg,
                        ins=[ag3b_in.opt()], outs=[ag3b_out.opt()])


                    if stage == 5:
                        dmy = sbF.tile([128, CH], F32, tag="dmy", name="dmy5")
                        nc.vector.memset(dmy[:], 0.0)
                        nc.sync.dma_start(scores[:], dmy[:])
                        return nc
                    # ---------- Phase F: pair gather + bilinear score ----------
                    # HW indirect DMA honors one index per partition: 64 calls
                    # of 128-row gathers per table, pipelined on the Q7/SDMA path
                    g1_sb = sbF.tile([128, CH, NHID], F32, tag="g1")
                    g2_sb = sbF.tile([128, CH, NHID], F32, tag="g2")
                    for c in range(CH):
                        nc.gpsimd.indirect_dma_start(
                            out=g1_sb[:, c, :], out_offset=None, in_=ag3a_out[:],
                            in_offset=bass.IndirectOffsetOnAxis(
                                ap=idx1_sb[:, c:c + 1], axis=0))
                        nc.gpsimd.indirect_dma_start(
                            out=g2_sb[:, c, :], out_offset=None, in_=ag3b_out[:],
                            in_offset=bass.IndirectOffsetOnAxis(
                                ap=idx2_sb[:, c:c + 1], axis=0))

                    if stage == 6:
                        dmy = sbF.tile([128, CH], F32, tag="dmy", name="dmy6")
                        nc.vector.memset(dmy[:], 0.0)
                        nc.sync.dma_start(scores[:], dmy[:])
                        return nc
                    prod_sb = sbF.tile([128, CH, NHID], F32, tag="prod")
                    nc.vector.tensor_tensor(prod_sb[:], g1_sb[:], g2_sb[:], OP.mult)
                    sc_sb = sbF.tile([128, CH], F32, tag="sc")
                    nc.vector.tensor_reduce(sc_sb[:], prod_sb[:], AX.X, OP.add)
                    nc.sync.dma_start(scores[:], sc_sb[:])

    return nc


def _make_in_maps(x, adj, W_heads, a_heads, W_out, a_out, W_score,
                  pair1_idx, pair2_idx):
    bf = ml_dtypes.bfloat16
    x = np.asarray(x, dtype=np.float32)
    adj = np.asarray(adj, dtype=np.float32)
    W_heads = np.asarray(W_heads, dtype=np.float32)
    a_heads = np.asarray(a_heads, dtype=np.float32)
    W_out = np.asarray(W_out, dtype=np.float32)
    a_out = np.asarray(a_out, dtype=np.float32)
    W_score = np.asarray(W_score, dtype=np.float32)
    pair1_idx = np.asarray(pair1_idx, dtype=np.int32)
    pair2_idx = np.asarray(pair2_idx, dtype=np.int32)

    Wcat = np.concatenate([W_heads[h] for h in range(NHEADS)], axis=1)
    Wcat = np.ascontiguousarray(Wcat, dtype=np.float32)
    Asrc = np.zeros((NHEADS * NHID, NHEADS), dtype=np.float32)
    Adst = np.zeros((NHEADS * NHID, NHEADS), dtype=np.float32)
    for h in range(NHEADS):
        Asrc[h * NHID:(h + 1) * NHID, h] = a_heads[h, :NHID]
        Adst[h * NHID:(h + 1) * NHID, h] = a_heads[h, NHID:]
    Wout_bf = W_out.astype(bf)
    aout2 = np.stack([a_out[:NHID], a_out[NHID:]], axis=1).astype(bf)
    WsT = np.ascontiguousarray(W_score.T, dtype=np.float32)
    ident = np.eye(128, dtype=np.float32)
    selbc = np.zeros((NHEADS, R), dtype=np.float32)
    for h in range(NHEADS):
        selbc[h, h * NHID:(h + 1) * NHID] = 1.0

    def fold(a, k):
        # [k*128, w] row-major -> [128, k*w] with row p holding blocks k
        return a.reshape(k, 128, a.shape[1]).transpose(1, 0, 2).reshape(128, -1)

    def pad128(a):
        out = np.zeros((128, a.shape[1]), a.dtype)
        out[:a.shape[0]] = a
        return out

    # replicated f32/bf16 blob pieces (everything except xT/maskT/idx)
    f32_rep = [fold(Wcat, 4), fold(Asrc, 4), fold(Adst, 4),
               pad128(WsT), ident, pad128(selbc)]
    b16_rep = [fold(Wout_bf, 4), pad128(aout2)]

    in_maps = []
    for c in range(NCORES):
        rows = slice(c * R, (c + 1) * R)
        xT = np.ascontiguousarray(x[rows].T)
        maskT = np.ascontiguousarray(adj[rows].T).astype(bf)
        blobF = np.concatenate([fold(xT, 4)] + f32_rep, axis=1)
        blobB = np.concatenate([fold(maskT, JT)] + b16_rep, axis=1)

        blobI = np.concatenate(
            [pair1_idx[c * PC:(c + 1) * PC].reshape(128, CH),
             pair2_idx[c * PC:(c + 1) * PC].reshape(128, CH)], axis=1)
        assert blobF.shape == (128, FW) and blobB.shape == (128, BW)
        in_maps.append(dict(
            blobF=np.ascontiguousarray(blobF),
            blobB=np.ascontiguousarray(blobB),
            blobI=np.ascontiguousarray(blobI),
        ))
    return in_maps


_CACHE = {}


def _get_compiled(stage=99, iters=1):
    key = f"nc{stage}_{iters}"
    if key not in _CACHE:
        nc = _build_nc(stage, iters)
        nc.compile()
        _CACHE[key] = nc
    return _CACHE[key]


def kernel(**inputs):
    from concourse.bass_utils import run_bass_kernel_spmd

    nc = _get_compiled()
    in_maps = _make_in_maps(**inputs)
    res = run_bass_kernel_spmd(nc, in_maps, core_ids=list(range(NCORES)))
    out = np.concatenate(
        [np.asarray(res.results[c]["scores"], dtype=np.float32).reshape(PC)
         for c in range(NCORES)])
    return out

